# revision 1
# baseline (speedup 1.0000x reference)
"""E8P codebook dequant kernel for 8x TRN2 NeuronCores (Bass/Tile).

Row-parallel sharding: core c handles rows [512c, 512c+512) of weight_q and
produces the matching [512, 11008] f32 slice of the output. grid and scale
are replicated to every core. No cross-core communication.

On-device algorithm (per core):
  - One SBUF table tensor T [128, 32768] f32 holds the scale-folded codebook,
    split across partition halves of each 16-partition GPSIMD group:
      partitions p with p%16 == j < 8:  T[p][s] = scale*grid[s-1][j]
                                        (s in 1..32767 -> entries 0..32766)
      partitions p with p%16 == 8+j:    T[p][s] = scale*grid[32766+s][j]
                                        (s in 1..32767 -> entries 32767..65533)
      slot 0 is 0.0 everywhere (sentinel; ap_gather clamps negative stream
      values to slot 0).
  - T3 [128, 4] covers the two remaining entries: slots 1,2 hold
    scale*grid[65534/65535][j] on low partitions, 0 elsewhere.
  - Index streams (int16, computed in u16 with wraparound then bitcast):
      s1 = idx + 1               valid for idx in [0, 32766]
      s2 = idx - 32766           valid for idx in [32767, 65533]
      s3 = max(idx, 65533) - 65533   -> 0 / 1 / 2
  - Three ap_gather calls per chunk over the shared tables; merge
      out[16g+j] = (X1 + X3)[16g+j] + X2[16g+8+j]
    where exactly one term is nonzero per element, so f32 adds are exact.
"""

import numpy as np

import concourse.bass as bass
import concourse.bacc as bacc
import concourse.tile as tile
import concourse.mybir as mybir
from concourse.bass_utils import run_bass_kernel_spmd

OUT_F = 4096
IN_F = 11008
CODESZ = 8
CB = 65536
N_CORES = 8

ROWS = OUT_F // N_CORES          # 512 rows per core
QCOLS = IN_F // CODESZ           # 1376 codes per row
N_IDX = ROWS * QCOLS             # 704512 indices per core
PER_PART = N_IDX // 128          # 5504 indices per partition (= 4 rows)

F_CHUNK = 344                    # 1376 = 4 * 344: chunks never cross a row
S_CHUNK = F_CHUNK * 16           # 5504 stream elements per group per call
N_CHUNKS = PER_PART // F_CHUNK   # 16
CHUNKS_PER_ROW = QCOLS // F_CHUNK  # 4

_CACHE: dict = {}
REPEAT = 1  # device-work multiplier (timing experiments only)


def _build():
    if "nc" in _CACHE:
        return _CACHE["nc"]
    dt = mybir.dt
    nc = bacc.Bacc("TRN2", target_bir_lowering=False, debug=False,
                   enable_asserts=False, num_devices=N_CORES,
                   dynamic_dma_scratch_size=2048)
    wq_d = nc.dram_tensor("wq", [ROWS, QCOLS], dt.int32, kind="ExternalInput")
    # grid arrives host-transposed [8, 65536] so table loads are contiguous
    grid_d = nc.dram_tensor("gridT", [CODESZ, CB], dt.float32, kind="ExternalInput")
    scale_d = nc.dram_tensor("scale", [1], dt.float32, kind="ExternalInput")
    out_d = nc.dram_tensor("out", [N_CHUNKS * 8 * 8 * F_CHUNK * 16],
                       dt.float32, kind="ExternalOutput")

    with tile.TileContext(nc) as tc:
        with tc.tile_pool(name="tab", bufs=1) as tabp, \
             tc.tile_pool(name="small", bufs=1) as smallp, \
             tc.tile_pool(name="idx", bufs=1) as idxp, \
             tc.tile_pool(name="st", bufs=1) as stp, \
             tc.tile_pool(name="x1", bufs=1) as x1p, \
             tc.tile_pool(name="x3", bufs=1) as x3p, \
             tc.tile_pool(name="xunused", bufs=1) as x2sp:

            # ---- scale broadcast to all 128 partitions ----
            scale_t = smallp.tile([128, 1], dt.float32)
            nc.sync.dma_start(scale_t[:], bass.AP(scale_d, 0, [[0, 128], [1, 1]]))

            # ---- codebook table T ----
            T = tabp.tile([128, 32768], dt.float32)
            for j in range(8):
                # low half: entries 0..32766 -> slots 1..32767
                nc.sync.dma_start(
                    T[:][j::16, 1:32768],
                    bass.AP(grid_d, j * CB, [[0, 8], [1, 32767]]),
                )
                # high half: entries 32767..65533 -> slots 1..32767
                nc.sync.dma_start(
                    T[:][(8 + j)::16, 1:32768],
                    bass.AP(grid_d, j * CB + 32767, [[0, 8], [1, 32767]]),
                )
            nc.vector.memset(T[:][:, 0:1], 0.0)
            # fold scale into the table (f32, same rounding as reference)
            nc.vector.tensor_scalar(T[:], T[:], scale_t[:], None,
                                    mybir.AluOpType.mult)

            # ---- T3 for entries 65534/65535: 768 slots so the mostly-zero
            #      stream spreads reads over 64 addresses (slot-conflict
            #      fix); s3 = (idx&63) + 64*fix -> homeless at 126, 191 ----
            T3 = smallp.tile([128, 192], dt.float32)
            nc.vector.memset(T3[:], 0.0)
            for j in range(8):
                nc.sync.dma_start(
                    T3[:][j::16, 126:127],
                    bass.AP(grid_d, j * CB + 65534, [[0, 8], [1, 1]]),
                )
                nc.sync.dma_start(
                    T3[:][j::16, 191:192],
                    bass.AP(grid_d, j * CB + 65535, [[0, 8], [1, 1]]),
                )
            nc.vector.tensor_scalar(T3[:], T3[:], scale_t[:], None,
                                    mybir.AluOpType.mult)

            add = mybir.AluOpType.add
            sub = mybir.AluOpType.subtract
            mx = mybir.AluOpType.max

            for u in [u for _ in range(REPEAT) for u in range(N_CHUNKS // 2)]:
                ta, tb = 2 * u, 2 * u + 1
                # load both chunks' codes up front for the paired T3 stream
                wq_a = stp.tile([128, F_CHUNK], dt.int32, tag="wqa")
                wq_b = stp.tile([128, F_CHUNK], dt.int32, tag="wqb")
                nc.sync.dma_start(
                    wq_a[:],
                    bass.AP(wq_d, ta * F_CHUNK, [[PER_PART, 128], [1, F_CHUNK]]))
                nc.sync.dma_start(
                    wq_b[:],
                    bass.AP(wq_d, tb * F_CHUNK, [[PER_PART, 128], [1, F_CHUNK]]))
                ida = wq_a[:].bitcast(dt.uint16)[:, 0::2]
                idb = wq_b[:].bitcast(dt.uint16)[:, 0::2]

                # one T3 gather covers both chunks (amortizes call overhead)
                s3p = stp.tile([128, 2 * F_CHUNK], dt.int16, tag="s3p")
                s3b = stp.tile([128, 2 * F_CHUNK], dt.int16, tag="s3b")
                band = mybir.AluOpType.bitwise_and
                shl = mybir.AluOpType.logical_shift_left
                addo = mybir.AluOpType.add
                for (idq, off) in ((ida, 0), (idb, F_CHUNK)):
                    sl = slice(off, off + F_CHUNK)
                    nc.vector.tensor_scalar(
                        s3b[:].bitcast(dt.uint16)[:, sl], idq, 63, None, band)
                    nc.vector.tensor_scalar(
                        s3p[:].bitcast(dt.uint16)[:, sl], idq,
                        65533, 65533, mx, sub)
                nc.vector.tensor_scalar(
                    s3p[:].bitcast(dt.uint16), s3p[:].bitcast(dt.uint16),
                    6, None, shl)
                nc.vector.tensor_tensor(
                    s3p[:].bitcast(dt.uint16), s3p[:].bitcast(dt.uint16),
                    s3b[:].bitcast(dt.uint16), addo)
                X3p = x3p.tile([128, 2 * S_CHUNK], dt.float32)
                nc.gpsimd.ap_gather(X3p[:], T3[:], s3p[:], channels=128,
                                    num_elems=192, d=1, num_idxs=2 * S_CHUNK)

                for (t, idc, x3off) in ((ta, ida, 0), (tb, idb, S_CHUNK)):
                    s12 = stp.tile([128, 2 * F_CHUNK], dt.int16, tag="s12")
                    nc.vector.tensor_scalar(
                        s12[:].bitcast(dt.uint16)[:, 0:F_CHUNK], idc, 1, None, add)
                    nc.vector.tensor_scalar(
                        s12[:].bitcast(dt.uint16)[:, F_CHUNK:], idc, 32766, None, sub)

                    X12 = x1p.tile([128, 2 * S_CHUNK], dt.float32)
                    nc.gpsimd.ap_gather(X12[:], T[:], s12[:], channels=128,
                                        num_elems=32768, d=1, num_idxs=2 * S_CHUNK)
                    X1 = X12[:][:, 0:S_CHUNK]
                    X2 = X12[:][:, S_CHUNK:2 * S_CHUNK]
                    X3c = X3p[:][:, x3off:x3off + S_CHUNK]

                    # in-place partition shift of the high half, then merge
                    shuf = [(8 + i) if (i % 16) < 8 else i for i in range(32)]
                    nc.vector.stream_shuffle(X2, X2, shuf)
                    nc.vector.tensor_add(X3c, X3c, X1)
                    nc.vector.tensor_add(X1, X3c, X2)

                    # ---- planar write back (same layout as before) ----
                    for j in range(8):
                        src_ap = X12[:][j::16, 0:S_CHUNK].rearrange(
                            "p (f pp) -> p f pp", pp=16)
                        blk = 8 * F_CHUNK * 16
                        dst = bass.AP(
                            out_d, (t * 8 + j) * blk,
                            [[F_CHUNK * 16, 8], [16, F_CHUNK], [1, 16]],
                        )
                        nc.sync.dma_start(dst, src_ap)

    nc.compile()
    _CACHE["nc"] = nc
    return nc


def kernel(weight_q: np.ndarray, grid: np.ndarray, scale: np.ndarray) -> np.ndarray:
    weight_q = np.ascontiguousarray(np.asarray(weight_q, dtype=np.int32))
    grid = np.ascontiguousarray(np.asarray(grid, dtype=np.float32))
    scale = np.ascontiguousarray(np.asarray(scale, dtype=np.float32))
    nc = _build()
    grid_t = np.ascontiguousarray(grid.T)   # layout marshalling for replication
    in_maps = []
    for c in range(N_CORES):
        in_maps.append({
            "wq": weight_q[c * ROWS:(c + 1) * ROWS],
            "gridT": grid_t,
            "scale": scale,
        })
    res = run_bass_kernel_spmd(nc, in_maps, core_ids=list(range(N_CORES)))
    shards = []
    for c in range(N_CORES):
        planar = res.results[c]["out"].reshape(N_CHUNKS, 8, 8, F_CHUNK, 16)
        # element (t, j, g, f, pp) -> row 64g + 4pp + t//8,
        #                            col ((t%8)*F_CHUNK + f)*8 + j
        p6 = planar.reshape(4, CHUNKS_PER_ROW, 8, 8, F_CHUNK, 16)  # tt, tq, j, g, f, pp
        # -> [g, pp, tt, tq, f, j]
        x = np.transpose(p6, (3, 5, 0, 1, 4, 2))
        shards.append(x.reshape(ROWS, IN_F))
    return np.concatenate(shards, axis=0)


if __name__ == "__main__":
    rng = np.random.default_rng(0)
    wq = rng.integers(0, CB, size=(OUT_F, QCOLS), dtype=np.int32)
    g = rng.standard_normal((CB, CODESZ)).astype(np.float32)
    s = rng.random(1).astype(np.float32)
    got = kernel(wq, g, s)
    exp = (g[wq].reshape(OUT_F, IN_F) * s).astype(np.float32)
    err = np.abs(got - exp)
    denom = np.maximum(np.abs(exp), 1e-6)
    print("max abs err:", err.max())
    print("max rel err:", (err / denom).max())
    print("exact match:", np.array_equal(got, exp))



# revision 2
# speedup vs baseline: 3.2056x; 3.2056x over previous
"""E8P codebook dequant kernel for 8x TRN2 NeuronCores (Bass/Tile), v2.

Row-parallel sharding: core c handles rows [512c, 512c+512) of weight_q and
produces the matching [512, 11008] f32 slice of the output. The codebook and
scale are replicated. No cross-core communication.

Strategy (per core): DMA-engine gather instead of GPSIMD ap_gather.
  - Host marshals the 2MB grid into a "pair table" [32768, 64] f32 (8MB):
    row k = [grid[2k] | grid[2k+1] | 48 f32 pad], so a 15-bit index k=idx>>1
    (int16-safe) fetches a 256B element containing both dequant candidates.
  - The 704512 codes are processed in 86 chunks of 8192. For each chunk the
    Pool engine issues one SWDGE dma_gather (8192 descriptors, 256B each,
    one per code) that lands G[p, j, 0:64] = tab[idx>>1] with code
    n = 8192*k + 64*p + j  <->  gather stream position i = 128*j + p
    (host pre-wraps the stream into the [16, 512]-replicated idx layout).
  - DVE selects the right half and applies scale in one pass:
    out = G[:, :, 0:8]*(s*(1-b0)) + G[:, :, 8:16]*(s*b0), b0 = idx & 1,
    with [128, 64] selector tiles broadcast (stride-0) over the 8 components.
    Exactly one term is nonzero, and x*s + 0 keeps reference f32 rounding.
  - The output tile [128, 512] f32 is written back contiguously: the
    partition-major code order makes the device buffer the row-major output.

DMA-bus cost per core: 86 * 2MB gathered + 22.5MB out + 12.4MB idx loads
~= 215MB at ~360GB/s -> ~0.6ms, ~12x under the ap_gather baseline.
"""

import numpy as np

import concourse.bass as bass
import concourse.bacc as bacc
import concourse.tile as tile
import concourse.mybir as mybir
from concourse.bass_utils import run_bass_kernel_spmd

OUT_F = 4096
IN_F = 11008
CODESZ = 8
CB = 65536
N_CORES = 8

ROWS = OUT_F // N_CORES          # 512 rows per core
QCOLS = IN_F // CODESZ           # 1376 codes per row
N_IDX = ROWS * QCOLS             # 704512 codes per core

C = 8192                         # codes per chunk
J = C // 128                     # 64 codes per partition per chunk
ELEM = 64                        # pair-table row: 64 f32 = 256B
TROWS = CB // 2                  # 32768 pair-table rows
N_CHUNKS = N_IDX // C            # 86

_CACHE: dict = {}
REPEAT = 1  # device-work multiplier (timing experiments only)


def _build():
    if "nc" in _CACHE:
        return _CACHE["nc"]
    dt = mybir.dt
    nc = bacc.Bacc("TRN2", target_bir_lowering=False, debug=False,
                   enable_asserts=False, num_devices=N_CORES,
                   num_swdge_queues=4)
    tab_d = nc.dram_tensor("tab", [TROWS, ELEM], dt.float32,
                           kind="ExternalInput")
    idxw_d = nc.dram_tensor("idxw", [N_CHUNKS * 128, C // 16], dt.int16,
                            kind="ExternalInput")
    idxp_d = nc.dram_tensor("idxp", [N_CHUNKS * 128, J], dt.int16,
                            kind="ExternalInput")
    scale_d = nc.dram_tensor("scale", [1], dt.float32, kind="ExternalInput")
    out_d = nc.dram_tensor("out", [N_CHUNKS * 128, J * CODESZ], dt.float32,
                           kind="ExternalOutput")

    mul = mybir.AluOpType.mult
    sub = mybir.AluOpType.subtract
    band = mybir.AluOpType.bitwise_and
    shr = mybir.AluOpType.logical_shift_right
    add = mybir.AluOpType.add

    with tile.TileContext(nc) as tc:
        with tc.tile_pool(name="small", bufs=1) as smallp, \
             tc.tile_pool(name="gath", bufs=3) as gp, \
             tc.tile_pool(name="idx", bufs=3) as ip, \
             tc.tile_pool(name="sel", bufs=3) as sp, \
             tc.tile_pool(name="outp", bufs=3) as op:

            scale_t = smallp.tile([128, 1], dt.float32)
            nc.sync.dma_start(scale_t[:], bass.AP(scale_d, 0, [[0, 128], [1, 1]]))

            for k in [k for _ in range(REPEAT) for k in range(N_CHUNKS)]:
                idxw_t = ip.tile([128, C // 16], dt.int16, tag="idxw")
                nc.sync.dma_start(
                    idxw_t[:], idxw_d.ap()[k * 128:(k + 1) * 128, :])
                idxp_t = ip.tile([128, J], dt.int16, tag="idxp")
                nc.sync.dma_start(
                    idxp_t[:], idxp_d.ap()[k * 128:(k + 1) * 128, :])

                # gather stream: idx >> 1 (15-bit pair-table row, int16-safe)
                sidx = ip.tile([128, C // 16], dt.int16, tag="sidx")
                nc.vector.tensor_scalar(sidx[:].bitcast(dt.uint16),
                                        idxw_t[:].bitcast(dt.uint16),
                                        1, None, shr)

                # the deployed SWDGE ring holds 64 descs/engine -> max 1024
                # idxs per dma_gather call; split the 8192-code group into 8
                G = gp.tile([128, J * ELEM], dt.float32, tag="G")
                G3 = G[:].rearrange("p (j e) -> p j e", e=ELEM)
                GSUB = 1024
                JSUB = GSUB // 128               # 8 codes/partition/call
                for g in range(C // GSUB):
                    nc.gpsimd.dma_gather(
                        G3[:, g * JSUB:(g + 1) * JSUB, :], tab_d.ap(),
                        sidx[:][:, g * (GSUB // 16):(g + 1) * (GSUB // 16)],
                        num_idxs=GSUB, num_idxs_reg=GSUB,
                        elem_size=ELEM, queue_num=(k * 8 + g) % 4)

                # selectors: s_hi = s*b0, s_lo = s*(1-b0) = (s_hi - s)*(-1)
                b0u = sp.tile([128, J], dt.uint16, tag="b0u")
                nc.vector.tensor_scalar(b0u[:], idxp_t[:].bitcast(dt.uint16),
                                        1, None, band)
                s_hi = sp.tile([128, J], dt.float32, tag="s_hi")
                nc.vector.tensor_scalar(s_hi[:], b0u[:], scale_t[:], None, mul)
                s_lo = sp.tile([128, J], dt.float32, tag="s_lo")
                nc.vector.tensor_scalar(s_lo[:], s_hi[:], scale_t[:], -1.0,
                                        sub, mul)

                t_lo = op.tile([128, J * CODESZ], dt.float32, tag="t_lo")
                out_t = op.tile([128, J * CODESZ], dt.float32, tag="out_t")
                s_lo_b = s_lo[:].unsqueeze(2).broadcast_to([128, J, CODESZ])
                s_hi_b = s_hi[:].unsqueeze(2).broadcast_to([128, J, CODESZ])
                t_lo3 = t_lo[:].rearrange("p (j e) -> p j e", e=CODESZ)
                out3 = out_t[:].rearrange("p (j e) -> p j e", e=CODESZ)
                nc.vector.tensor_tensor(t_lo3, G3[:, :, 0:CODESZ], s_lo_b, mul)
                nc.vector.tensor_tensor(out3, G3[:, :, CODESZ:2 * CODESZ],
                                        s_hi_b, mul)
                nc.vector.tensor_tensor(out_t[:], out_t[:], t_lo[:], add)
                nc.sync.dma_start(
                    out_d.ap()[k * 128:(k + 1) * 128, :], out_t[:])

    nc.compile()
    _CACHE["nc"] = nc
    return nc


GSUB = 1024
JSUB = GSUB // 128
NSUB = C // GSUB


def _marshal_core(idx_u16: np.ndarray):
    """idx_u16: flat [N_IDX] uint16 codes of one core's rows.
    Returns (idxw [N_CHUNKS*128, C/16] i16, idxp [N_CHUNKS*128, J] i16).

    Sub-gather g of chunk k covers codes n = 8192k + 64p + 8g + jj at stream
    position i = 128*jj + p, wrapped into idxw columns [64g, 64g+64) as
    (i%16, i//16), replicated across the 8 partition groups."""
    blk = idx_u16.reshape(N_CHUNKS, 128, J)              # [k, p, j]
    idxp = blk.reshape(N_CHUNKS * 128, J)
    a = blk.reshape(N_CHUNKS, 128, NSUB, JSUB)           # [k, p, g, jj]
    st = a.transpose(0, 2, 3, 1).reshape(N_CHUNKS, NSUB, GSUB)  # stream_g
    wr = st.reshape(N_CHUNKS, NSUB, GSUB // 16, 16).transpose(0, 1, 3, 2)
    cols = wr.transpose(0, 2, 1, 3).reshape(N_CHUNKS, 16, C // 16)  # [k,q,(g s)]
    idxw = np.broadcast_to(cols[:, None, :, :],
                           (N_CHUNKS, 8, 16, C // 16))
    idxw = idxw.reshape(N_CHUNKS * 128, C // 16)
    return (np.ascontiguousarray(idxw).view(np.int16),
            np.ascontiguousarray(idxp).view(np.int16))


def kernel(weight_q: np.ndarray, grid: np.ndarray, scale: np.ndarray) -> np.ndarray:
    weight_q = np.asarray(weight_q, dtype=np.int32)
    grid = np.ascontiguousarray(np.asarray(grid, dtype=np.float32))
    scale = np.ascontiguousarray(np.asarray(scale, dtype=np.float32))
    nc = _build()

    tab = np.zeros((TROWS, ELEM), np.float32)
    tab[:, 0:CODESZ] = grid[0::2]
    tab[:, CODESZ:2 * CODESZ] = grid[1::2]

    idx_all = weight_q.astype(np.uint16).reshape(N_CORES, N_IDX)
    in_maps = []
    for c in range(N_CORES):
        idxw, idxp = _marshal_core(idx_all[c])
        in_maps.append({"tab": tab, "idxw": idxw, "idxp": idxp,
                        "scale": scale})
    res = run_bass_kernel_spmd(nc, in_maps, core_ids=list(range(N_CORES)))
    shards = [res.results[c]["out"].reshape(ROWS, IN_F)
              for c in range(N_CORES)]
    return np.concatenate(shards, axis=0)


if __name__ == "__main__":
    rng = np.random.default_rng(0)
    wq = rng.integers(0, CB, size=(OUT_F, QCOLS), dtype=np.int32)
    g = rng.standard_normal((CB, CODESZ)).astype(np.float32)
    s = rng.random(1).astype(np.float32)
    got = kernel(wq, g, s)
    exp = (g[wq].reshape(OUT_F, IN_F) * s).astype(np.float32)
    err = np.abs(got - exp)
    denom = np.maximum(np.abs(exp), 1e-6)
    print("max abs err:", err.max())
    print("max rel err:", (err / denom).max())
    print("exact match:", np.array_equal(got, exp))
